# revision 1
# baseline (speedup 1.0000x reference)
"""GatedRGCN Trainium2 kernel — 8-core SPMD.

Sharding: core c owns graph c (nodes [256c, 256c+256)). All tensors live on
device in "T layout" (features on partitions, nodes on free dim).

Per layer:
  - xr_r = x_blk @ W_r (per-core src block only)          [PE]
  - partial agg over ALL dst via dense scaled adjacency:
      aggT[f, dst] += xr_r[src,f]^T @ McT_r[src, dst]     [PE]
  - grouped ReduceScatter(sum) over 8 cores -> own slice  [CC, overlaps agg]
  - h = relu(agg + W_root^T x + b)                        [DVE/ACT]
Gate (per-core, own graph only):
  - hg/qg via PE; qi = sum_l sigmoid(hg+qg_l)*q_l via tensor_scalar adds,
    one big ACT sigmoid per l-chunk, per-l multiplies split DVE/GPSIMD,
    binary-tree l-sum on DVE.
  - alpha via PE + ACT sigmoid; h' = alpha*tanh(qi) + (1-alpha)*h.

The dense adjacency McT_r[src, dst] = count_r(dst,src)/max(indeg_r(dst),1)
is built on host from the edge lists (integer preprocessing), so the mean
aggregation becomes two dense matmuls on the PE.
"""
import sys

for _p in ("/opt/trn_rl_repo", "/root/.axon_site/_ro/trn_rl_repo"):
    if _p not in sys.path:
        sys.path.append(_p)

import numpy as np
import concourse.bacc as bacc
import concourse.mybir as mybir
from concourse import tile
from concourse.bass_utils import run_bass_kernel_spmd

F32 = mybir.dt.float32
BF16 = mybir.dt.bfloat16
AF = mybir.ActivationFunctionType

N_CORES = 8
F = 768
FT = 6           # feature tiles of 128
BN = 2048        # total nodes
NB = 256         # nodes per core/graph
L = 64           # queries per graph
R = 3            # relations
DCH = 4          # dst chunks of 512 in the agg matmul
LCH = 2          # l-chunks of 32 in the gate
LC = L // LCH    # 32
GPS_MUL = 0     # of the LC multiplies per chunk, how many go to GPSIMD
ACT_L = 14       # of the LC adds per chunk, how many use the ACT sigmoid-bias path
FT_G = 6         # f-tiles per reduce-scatter group
NG = FT // FT_G  # number of RS groups per layer

_CACHE = {}


def _build(cc=True):
    nc = bacc.Bacc("TRN2", target_bir_lowering=False, debug=False,
                   num_devices=N_CORES)

    # ---- per-core external inputs ----
    xT = nc.dram_tensor("xT", [F, NB], BF16, kind="ExternalInput")
    mt = nc.dram_tensor("mt", [R + 1, NB, BN], BF16, kind="ExternalInput")
    wrel1 = nc.dram_tensor("wrel1", [R, F, F], BF16, kind="ExternalInput")
    wroot1 = nc.dram_tensor("wroot1", [F, F], BF16, kind="ExternalInput")
    wrel2 = nc.dram_tensor("wrel2", [R, F, F], BF16, kind="ExternalInput")
    wroot2 = nc.dram_tensor("wroot2", [F, F], BF16, kind="ExternalInput")
    wga = nc.dram_tensor("wga", [F, F], BF16, kind="ExternalInput")
    wgb = nc.dram_tensor("wgb", [F, F], BF16, kind="ExternalInput")
    wqa = nc.dram_tensor("wqa", [F, F], BF16, kind="ExternalInput")
    wqb = nc.dram_tensor("wqb", [F, F], BF16, kind="ExternalInput")
    qT = nc.dram_tensor("qT", [F, L], BF16, kind="ExternalInput")
    qTf = nc.dram_tensor("qTf", [F, L], F32, kind="ExternalInput")
    # biases packed [4, F]: rows = b1, b2, bg, bq
    bias = nc.dram_tensor("bias", [4, F], F32, kind="ExternalInput")
    outT = nc.dram_tensor("outT", [F, NB], F32, kind="ExternalOutput")

    with tile.TileContext(nc) as tc:
        with (
            tc.tile_pool(name="const", bufs=1) as cpool,      # weights etc.
            tc.tile_pool(name="wlayer", bufs=1) as wpool,     # per-layer W
            tc.tile_pool(name="state", bufs=1) as hpool,      # h tensors
            tc.tile_pool(name="big", bufs=1) as big,          # per-layer tensors
            tc.tile_pool(name="work", bufs=3) as wk,          # small work tiles
            tc.tile_pool(name="stage", bufs=2) as stpool,     # gate staging
            tc.tile_pool(name="ps", bufs=6, space="PSUM") as psp,
            tc.tile_pool(name="dram", bufs=2 * NG, space="DRAM") as dram,
        ):
            def new_ps(n=512):
                ps_t = psp.tile([128, 512], F32, tag="ps")
                return ps_t[:] if n == 512 else ps_t[:, :n]

            # ---- load constants ----
            xT_sb = cpool.tile([128, FT, NB], BF16, tag="xT")
            nc.sync.dma_start(xT_sb[:], xT[:].rearrange("(t p) n -> p t n", p=128))
            mt_sb = cpool.tile([128, R + 1, 2, BN], BF16, tag="mt")
            nc.sync.dma_start(mt_sb[:], mt[:].rearrange("r (s p) d -> p r s d", p=128))
            qT_sb = cpool.tile([128, FT, L], BF16, tag="qT")
            nc.sync.dma_start(qT_sb[:], qT[:].rearrange("(t p) l -> p t l", p=128))
            qTf_sb = cpool.tile([128, FT, L], F32, tag="qTf")
            nc.sync.dma_start(qTf_sb[:], qTf[:].rearrange("(t p) l -> p t l", p=128))
            bias_sb = cpool.tile([128, 4, FT], F32, tag="bias")
            nc.sync.dma_start(bias_sb[:], bias[:].rearrange("b (t p) -> p b t", p=128))

            def load_w(handle, tag):
                t = cpool.tile([128, FT, F], BF16, tag=tag)
                nc.sync.dma_start(t[:], handle[:].rearrange("(t p) f -> p t f", p=128))
                return t

            wga_sb = load_w(wga, "wga")
            wgb_sb = load_w(wgb, "wgb")
            wqa_sb = load_w(wqa, "wqa")
            wqb_sb = load_w(wqb, "wqb")

            out_f32 = None
            cur_bf = xT_sb  # [128, FT, NB] bf16 input to layer-1 matmuls

            for layer in range(2):
                wrel_h = wrel1 if layer == 0 else wrel2
                wroot_h = wroot1 if layer == 0 else wroot2
                wrel_sb = wpool.tile([128, R, FT, F], BF16, tag="wrel")
                nc.sync.dma_start(
                    wrel_sb[:], wrel_h[:].rearrange("r (t p) f -> p r t f", p=128))
                wroot_sb = wpool.tile([128, FT, F], BF16, tag="wroot")
                nc.sync.dma_start(
                    wroot_sb[:], wroot_h[:].rearrange("(t p) f -> p t f", p=128))

                # ---- xr_r = x_blk @ W_r  -> [src, fout] bf16 ----
                xr_sb = big.tile([128, R + 1, 2, F], BF16, tag="xr")
                for r in range(R + 1):
                    for s in range(2):
                        for fc in range(2):  # fout chunks of 384
                            ps = new_ps(384)
                            for k in range(FT):
                                w_ap = (wrel_sb[:, r, k, fc * 384:(fc + 1) * 384]
                                        if r < R else
                                        wroot_sb[:, k, fc * 384:(fc + 1) * 384])
                                nc.tensor.matmul(
                                    ps[:],
                                    cur_bf[:, k, s * 128:(s + 1) * 128],
                                    w_ap,
                                    start=(k == 0), stop=(k == FT - 1))
                            nc.any.tensor_copy(
                                xr_sb[:, r, s, fc * 384:(fc + 1) * 384], ps[:])

                # ---- partial agg over all dst, grouped reduce-scatter ----
                rs_sb = big.tile([128, FT, NB], F32, tag="rs_sb")
                for g in range(NG):
                    partial = dram.tile([N_CORES, FT_G * 128, NB], F32,
                                        tag="partial")
                    for lft in range(FT_G):
                        ft = g * FT_G + lft
                        for dc in range(DCH):
                            ps = new_ps()
                            first = True
                            for r in range(R + 1):
                                for s in range(2):
                                    last = (r == R and s == 1)
                                    nc.tensor.matmul(
                                        ps[:],
                                        xr_sb[:, r, s, ft * 128:(ft + 1) * 128],
                                        mt_sb[:, r, s, dc * 512:(dc + 1) * 512],
                                        start=first, stop=last)
                                    first = False
                            agg_sb = wk.tile([128, 512], F32, tag="agg_sb")
                            nc.any.tensor_copy(agg_sb[:], ps[:])
                            for j in range(2):
                                blk = 2 * dc + j
                                nc.sync.dma_start(
                                    partial[blk, lft * 128:(lft + 1) * 128, :],
                                    agg_sb[:, j * 256:(j + 1) * 256])
                    rs_out = dram.tile([FT_G * 128, NB], F32, tag="rs_out")
                    if cc:
                        nc.gpsimd.collective_compute(
                            "ReduceScatter", mybir.AluOpType.add,
                            replica_groups=[list(range(N_CORES))],
                            ins=[partial.opt()], outs=[rs_out.opt()])
                    else:
                        nc.sync.dma_start(rs_out[:], partial[0])
                    nc.sync.dma_start(
                        rs_sb[:, g * FT_G:(g + 1) * FT_G, :],
                        rs_out[:].rearrange("(t p) n -> p t n", p=128))

                # ---- h = relu(agg + root + b) ----
                # layer 0: bf16 only (feeds matmuls + bf16 gate chain)
                # layer 1: f32 (+bf16 copy) — f32 needed for the output chain
                hb = hpool.tile([128, FT, NB], BF16, tag="hb")
                hf = None
                if layer == 1:
                    hf = hpool.tile([128, FT, NB], F32, tag="h")
                for ft in range(FT):
                    if layer == 0:
                        nc.scalar.activation(hb[:, ft, :], rs_sb[:, ft, :], AF.Relu,
                                             bias=bias_sb[:, layer, ft:ft + 1])
                    else:
                        nc.scalar.activation(hf[:, ft, :], rs_sb[:, ft, :], AF.Relu,
                                             bias=bias_sb[:, layer, ft:ft + 1])
                        nc.gpsimd.tensor_copy(hb[:, ft, :], hf[:, ft, :])

                # ================= gate =================
                # hg / qg
                hg_bf = big.tile([128, FT, NB], BF16, tag="hg")
                for ft in range(FT):
                    ps = new_ps(NB)
                    for k in range(FT):
                        nc.tensor.matmul(
                            ps[:], wga_sb[:, k, ft * 128:(ft + 1) * 128],
                            hb[:, k, :], start=(k == 0), stop=(k == FT - 1))
                    nc.any.tensor_copy(hg_bf[:, ft, :], ps[:])
                qg_f = big.tile([128, FT, L], F32, tag="qg")
                for ft in range(FT):
                    ps = new_ps(L)
                    for k in range(FT):
                        nc.tensor.matmul(
                            ps[:], wgb_sb[:, k, ft * 128:(ft + 1) * 128],
                            qT_sb[:, k, :], start=(k == 0), stop=(k == FT - 1))
                    nc.scalar.activation(qg_f[:, ft, :], ps[:], AF.Identity,
                                         bias=bias_sb[:, 2, ft:ft + 1])

                # qi = sum_l sigmoid(hg + qg_l) * q_l
                qi_bf = big.tile([128, FT, NB], BF16, tag="qi")
                for ft in range(FT):
                    chunks = []
                    for ch in range(LCH):
                        st = stpool.tile([128, LC, NB], BF16, tag="stage")
                        nd = LC - ACT_L  # l's whose add runs on DVE
                        for l in range(nd):
                            gl = ch * LC + l
                            nc.vector.tensor_scalar_add(
                                st[:, l, :], hg_bf[:, ft, :],
                                qg_f[:, ft, gl:gl + 1])
                        nc.scalar.activation(st[:, 0:nd, :], st[:, 0:nd, :],
                                             AF.Sigmoid)
                        for l in range(nd, LC):
                            gl = ch * LC + l
                            nc.scalar.activation(
                                st[:, l, :], hg_bf[:, ft, :], AF.Sigmoid,
                                bias=qg_f[:, ft, gl:gl + 1])
                        for l in range(LC):
                            gl = ch * LC + l
                            eng = nc.gpsimd if l < GPS_MUL else nc.vector
                            eng.tensor_scalar_mul(
                                st[:, l, :], st[:, l, :],
                                qTf_sb[:, ft, gl:gl + 1])
                        half = LC // 2
                        while half >= 1:
                            nc.vector.tensor_add(
                                st[:, 0:half, :], st[:, 0:half, :],
                                st[:, half:2 * half, :])
                            half //= 2
                        chunks.append(st)
                    nc.vector.tensor_add(qi_bf[:, ft, :], chunks[0][:, 0, :],
                                         chunks[1][:, 0, :])

                # alpha = sigmoid(WqA h + WqB qi + bq); h' = h + alpha*(tanh(qi)-h)
                gb = hpool.tile([128, FT, NB], BF16, tag="gb")
                gf = None
                if layer == 1:
                    gf = hpool.tile([128, FT, NB], F32, tag="g")
                for ft in range(FT):
                    ps = new_ps(NB)
                    for k in range(FT):
                        nc.tensor.matmul(
                            ps[:], wqa_sb[:, k, ft * 128:(ft + 1) * 128],
                            hb[:, k, :], start=(k == 0), stop=False)
                    for k in range(FT):
                        nc.tensor.matmul(
                            ps[:], wqb_sb[:, k, ft * 128:(ft + 1) * 128],
                            qi_bf[:, k, :], start=False, stop=(k == FT - 1))
                    if layer == 0:
                        al = wk.tile([128, NB], BF16, tag="alpha")
                        nc.scalar.activation(al[:], ps[:], AF.Sigmoid,
                                             bias=bias_sb[:, 3, ft:ft + 1])
                        th = wk.tile([128, NB], BF16, tag="tanh")
                        nc.scalar.activation(th[:], qi_bf[:, ft, :], AF.Tanh)
                        nc.vector.tensor_sub(th[:], th[:], hb[:, ft, :])
                        nc.vector.tensor_mul(th[:], th[:], al[:])
                        nc.vector.tensor_add(gb[:, ft, :], hb[:, ft, :], th[:])
                    else:
                        al = wk.tile([128, NB], F32, tag="alphaf")
                        nc.scalar.activation(al[:], ps[:], AF.Sigmoid,
                                             bias=bias_sb[:, 3, ft:ft + 1])
                        th = wk.tile([128, NB], F32, tag="tanhf")
                        nc.scalar.activation(th[:], qi_bf[:, ft, :], AF.Tanh)
                        nc.vector.tensor_sub(th[:], th[:], hf[:, ft, :])
                        nc.vector.tensor_mul(th[:], th[:], al[:])
                        nc.vector.tensor_add(gf[:, ft, :], hf[:, ft, :], th[:])
                out_f32 = gf
                cur_bf = gb

            # ---- output ----
            nc.sync.dma_start(
                outT[:].rearrange("(t p) n -> p t n", p=128), out_f32[:])

    nc.compile()
    return nc


def _preprocess(x, edge_index, edge_type, query_embs,
                W_rel1, W_root1, b1, W_rel2, W_root2, b2, Wg, bg, Wq, bq):
    x = np.asarray(x, np.float32)
    ei = np.asarray(edge_index).astype(np.int64)
    et = np.asarray(edge_type).astype(np.int64)
    q = np.asarray(query_embs, np.float32)

    src, dst = ei[0], ei[1]
    mc = np.zeros((R + 1, BN, BN), np.float32)
    np.add.at(mc, (et, dst, src), 1.0)
    cnt = mc[:R].sum(axis=2)
    mc[:R] /= np.maximum(cnt, 1.0)[:, :, None]
    mc[R] = np.eye(BN, dtype=np.float32)  # identity slab carries W_root
    mcT = np.ascontiguousarray(mc.transpose(0, 2, 1))  # [R+1, src, dst]

    def bf(a):
        import ml_dtypes
        return np.asarray(a, np.float32).astype(ml_dtypes.bfloat16)

    xT = np.ascontiguousarray(np.asarray(x).T)  # [F, BN]
    bias = np.stack([np.asarray(b1, np.float32), np.asarray(b2, np.float32),
                     np.asarray(bg, np.float32), np.asarray(bq, np.float32)])

    shared = {
        "wrel1": bf(W_rel1), "wroot1": bf(W_root1),
        "wrel2": bf(W_rel2), "wroot2": bf(W_root2),
        "wga": bf(np.asarray(Wg, np.float32)[:, :F].T),
        "wgb": bf(np.asarray(Wg, np.float32)[:, F:].T),
        "wqa": bf(np.asarray(Wq, np.float32)[:, :F].T),
        "wqb": bf(np.asarray(Wq, np.float32)[:, F:].T),
        "bias": bias,
    }
    in_maps = []
    for c in range(N_CORES):
        nb = slice(NB * c, NB * (c + 1))
        m = dict(shared)
        m["xT"] = bf(xT[:, nb])
        m["mt"] = bf(mcT[:, nb, :])
        m["qT"] = bf(q[c].T)
        m["qTf"] = np.ascontiguousarray(q[c].T)
        in_maps.append(m)
    return in_maps


def kernel(**inputs):
    if "nc" not in _CACHE:
        _CACHE["nc"] = _build()
    nc = _CACHE["nc"]
    in_maps = _preprocess(**inputs)
    res = run_bass_kernel_spmd(nc, in_maps, list(range(N_CORES)))
    out = np.concatenate(
        [np.asarray(res.results[c]["outT"], np.float32).T for c in range(N_CORES)],
        axis=0)
    return out

